# revision 2
# baseline (speedup 1.0000x reference)
"""Trainium2 Bass kernel for GCAFA block (conv1x1+BN+PReLU -> axial W attention
-> proj conv + residual -> gated conv + residual).

Sharding: batch B=8 across 8 NeuronCores (data parallel), params replicated.
All matmuls in bf16 with fp32 PSUM accumulation; output fp32.

v3: 4-row (2-pair) groups to batch ACT work; exp in one ACT per pair;
v-PReLU on DVE; cast + final residual add on GpSimd; persistent
ones-interleaved V^T buffers.
"""

import os
import sys

for _p in ("/opt/trn_rl_repo", "/root/.axon_site/_ro/trn_rl_repo"):
    if os.path.isdir(_p) and _p not in sys.path:
        sys.path.insert(0, _p)

import numpy as np
import ml_dtypes

import concourse.bacc as bacc
import concourse.tile as tile
from concourse import mybir
from concourse.bass_utils import run_bass_kernel_spmd

B, C, H, W = 8, 128, 224, 224
CA = C // 2  # 64
EPS = 1e-5
N_CORES = 8
PIX = H * W

F32 = mybir.dt.float32
BF = mybir.dt.bfloat16
AF = mybir.ActivationFunctionType
ALU = mybir.AluOpType

_CACHE = {}

N_VTS = 8  # persistent rotating V^T buffers (ones pre-interleaved)


def build(n_pairs=H // 2, debug_dump=False):
    """Build + compile the per-core Bass program processing 2*n_pairs rows."""
    assert n_pairs % 2 == 0
    n_groups = n_pairs // 2
    nc = bacc.Bacc("TRN2", target_bir_lowering=False, debug=False,
                   num_devices=N_CORES)
    npx = n_pairs * 2 * W  # pixels processed

    x_d = nc.dram_tensor("x", [C, npx], F32, kind="ExternalInput").ap()
    out_d = nc.dram_tensor("out", [C, npx], F32, kind="ExternalOutput").ap()
    wq_d = nc.dram_tensor("wq", [C, CA], BF, kind="ExternalInput").ap()
    wk_d = nc.dram_tensor("wk", [C, CA], BF, kind="ExternalInput").ap()
    wv_d = nc.dram_tensor("wv", [C, CA], BF, kind="ExternalInput").ap()
    wp_d = nc.dram_tensor("wp", [CA + 1, C], BF, kind="ExternalInput").ap()
    wg_d = nc.dram_tensor("wg", [C, C], BF, kind="ExternalInput").ap()
    bq_d = nc.dram_tensor("bq", [C, 1], F32, kind="ExternalInput").ap()
    bk_d = nc.dram_tensor("bk", [C, 1], F32, kind="ExternalInput").ap()
    bv_d = nc.dram_tensor("bv", [C, 1], F32, kind="ExternalInput").ap()
    b2_d = nc.dram_tensor("b2", [C, 1], F32, kind="ExternalInput").ap()
    b3_d = nc.dram_tensor("b3", [C, 1], F32, kind="ExternalInput").ap()
    id_d = nc.dram_tensor("ident", [C, C], BF, kind="ExternalInput").ap()

    W2 = 2 * W    # 448 = one pair of rows
    W4 = 4 * W    # 896 = one group (2 pairs)

    with tile.TileContext(nc) as tc:
        with (
            tc.tile_pool(name="consts", bufs=1) as cpool,
            tc.tile_pool(name="io", bufs=3) as iop,
            tc.tile_pool(name="acts", bufs=2) as ap_,
            tc.tile_pool(name="attn", bufs=3) as atp,
            tc.tile_pool(name="outs", bufs=3) as otp,
            tc.tile_pool(name="ps_qkv", bufs=1, space="PSUM") as ps_qkv,
            tc.tile_pool(name="ps_st", bufs=1, space="PSUM") as ps_st,
            tc.tile_pool(name="ps_vt", bufs=1, space="PSUM") as ps_vt,
            tc.tile_pool(name="ps_o", bufs=1, space="PSUM") as ps_o,
            tc.tile_pool(name="ps_pg", bufs=1, space="PSUM") as ps_pg,
        ):
            # ---- constants (loaded once) ----
            wq = cpool.tile([C, CA], BF, tag="wq")
            wk = cpool.tile([C, CA], BF, tag="wk")
            wv = cpool.tile([C, CA], BF, tag="wv")
            wp = cpool.tile([CA + 1, C], BF, tag="wp")
            wg = cpool.tile([C, C], BF, tag="wg")
            bq = cpool.tile([C, 1], F32, tag="bq")
            bk = cpool.tile([C, 1], F32, tag="bk")
            bv = cpool.tile([C, 1], F32, tag="bv")
            b2 = cpool.tile([C, 1], F32, tag="b2")
            b3 = cpool.tile([C, 1], F32, tag="b3")
            ident = cpool.tile([C, C], BF, tag="id")
            for t, d in ((wq, wq_d), (wk, wk_d), (wv, wv_d), (wp, wp_d),
                         (wg, wg_d), (bq, bq_d), (bk, bk_d), (bv, bv_d),
                         (b2, b2_d), (b3, b3_d), (ident, id_d)):
                nc.sync.dma_start(t[:], d[:])

            # persistent V^T buffers: per 66-col chunk [pad, ones, V^T(64)];
            # memset to 1.0 once, V-slots overwritten each use, ones survive.
            vts_bufs = []
            for i in range(N_VTS):
                vt_t = cpool.tile([112, 132], BF, tag=f"vts{i}")
                nc.gpsimd.memset(vt_t[:], 1.0)
                vts_bufs.append(vt_t)

            for g in range(n_groups):
                c0 = g * W4
                # ---- load + cast input group (4 rows) ----
                xf = iop.tile([C, W4], F32, tag="xf")
                nc.sync.dma_start(xf[:], x_d[:, c0:c0 + W4])
                xb = iop.tile([C, W4], BF, tag="xb")
                nc.gpsimd.tensor_copy(xb[:], xf[:])

                # ---- qkv convs for the group ----
                # tiles [128, 448]: partitions = 2 rows x 64 chans
                # (row even -> parts 0:64, row odd -> 64:128),
                # cols = pair index * 224 + pixel
                q_ps = ps_qkv.tile([C, W2], F32, tag="q")
                k_ps = ps_qkv.tile([C, W2], F32, tag="k")
                v_ps = ps_qkv.tile([C, W2], F32, tag="v")
                for wmat, dst in ((wq, q_ps), (wk, k_ps), (wv, v_ps)):
                    for p in range(2):
                        for r in range(2):
                            rs = slice((2 * p + r) * W, (2 * p + r + 1) * W)
                            od = slice(r * CA, (r + 1) * CA)
                            cs = slice(p * W, (p + 1) * W)
                            nc.tensor.matmul(dst[od, cs], wmat[:], xb[:, rs],
                                             start=True, stop=True,
                                             tile_position=(0, r * CA))
                # q/k PReLU on ACT (one instr per tensor, whole group)
                qsb = ap_.tile([C, W2], BF, tag="q")
                ksb = ap_.tile([C, W2], BF, tag="k")
                nc.scalar.activation(qsb[:], q_ps[:], AF.Prelu,
                                     bias=bq[:], scale=1.0, alpha=0.25)
                nc.scalar.activation(ksb[:], k_ps[:], AF.Prelu,
                                     bias=bk[:], scale=1.0, alpha=0.25)
                # v PReLU on DVE: t = v + bv; A = relu(t); B = t - A = min(t,0)
                # vsb = 0.25*B + A
                va = ap_.tile([C, W2], BF, tag="va")
                vb = ap_.tile([C, W2], BF, tag="vb")
                vsb = ap_.tile([C, W2], BF, tag="v")
                nc.vector.tensor_scalar(va[:], v_ps[:], bv[:], 0.0,
                                        op0=ALU.add, op1=ALU.max)
                nc.vector.scalar_tensor_tensor(vb[:], v_ps[:], bv[:], va[:],
                                               op0=ALU.add, op1=ALU.subtract)
                nc.vector.scalar_tensor_tensor(vsb[:], vb[:], 0.25, va[:],
                                               op0=ALU.mult, op1=ALU.add)

                for p in range(2):
                    pr = 2 * g + p          # global pair index
                    po = p * W              # pair col offset in group tiles
                    pc = c0 + p * W2        # pair col offset in DRAM

                    # ---- scores: st [112, 1024]; row h at cols 0:448,
                    # row h+1 at cols 512:960 (bank-aligned chunks) ----
                    st = ps_st.tile([112, 1024], F32, tag="st")
                    for r in range(2):
                        part = slice(r * CA, (r + 1) * CA)
                        ro = r * 512
                        nc.tensor.matmul(
                            st[:, ro:ro + W], ksb[part, po:po + 112],
                            qsb[part, po:po + W],
                            start=True, stop=True, tile_position=(r * CA, 0))
                        nc.tensor.matmul(
                            st[:, ro + W:ro + W2], ksb[part, po + 112:po + W],
                            qsb[part, po:po + W],
                            start=True, stop=True, tile_position=(r * CA, 0))
                    # exp over the whole tile (junk cols harmless)
                    esb = atp.tile([112, 1024], BF, tag="e")
                    nc.scalar.activation(esb[:], st[:], AF.Exp,
                                         bias=0.0, scale=0.125)

                    # ---- V^T via PE transpose + interleave copy ----
                    vt_ps = ps_vt.tile([112, 2 * C], BF, tag="vt")
                    nc.tensor.transpose(vt_ps[:, 0:C],
                                        vsb[:, po:po + 112], ident[:])
                    nc.tensor.transpose(vt_ps[:, C:2 * C],
                                        vsb[:, po + 112:po + W], ident[:])
                    vtg = vt_ps[:].rearrange("p (c x) -> p c x", x=CA)
                    vts0 = vts_bufs[(2 * pr) % N_VTS]
                    vts1 = vts_bufs[(2 * pr + 1) % N_VTS]
                    for r, vts in ((0, vts0), (1, vts1)):
                        vtv = vts[:].rearrange("p (c x) -> p c x", x=CA + 2)
                        nc.vector.tensor_copy(vtv[:, :, 2:CA + 2],
                                              vtg[:, r::2, :])

                    # ---- PV: O' [65, 448]; row 0 = softmax denominator ----
                    o_ps = ps_o.tile([CA + 1, W2], F32, tag="o")
                    for r, vts in ((0, vts0), (1, vts1)):
                        ow = slice(r * W, (r + 1) * W)
                        eo = r * 512
                        nc.tensor.matmul(o_ps[:, ow], vts[:, 1:CA + 2],
                                         esb[:, eo:eo + W],
                                         start=True, stop=False)
                        nc.tensor.matmul(o_ps[:, ow], vts[:, CA + 3:2 * CA + 4],
                                         esb[:, eo + W:eo + W2],
                                         start=False, stop=True)

                    # ---- normalize: obar = O' * (1/denom) ----
                    rden = atp.tile([1, W2], F32, tag="rden")
                    nc.vector.reciprocal_approx_fast(rden[:], o_ps[0:1, :])
                    rbc = atp.tile([CA + 1, W2], F32, tag="rbc")
                    nc.gpsimd.partition_broadcast(rbc[:], rden[:])
                    obar = atp.tile([CA + 1, W2], BF, tag="obar")
                    nc.vector.tensor_tensor(obar[:], o_ps[:], rbc[:], ALU.mult)

                    # ---- proj conv + BN + PReLU + residual ----
                    pj_ps = ps_pg.tile([C, W2], F32, tag="pg")
                    nc.tensor.matmul(pj_ps[:], wp[:], obar[:],
                                     start=True, stop=True)
                    t1 = otp.tile([C, W2], BF, tag="t1")
                    nc.scalar.activation(t1[:], pj_ps[:], AF.Prelu,
                                         bias=b2[:], scale=1.0, alpha=0.25)
                    out1 = otp.tile([C, W2], BF, tag="out1")
                    nc.vector.tensor_tensor(out1[:], t1[:],
                                            xb[:, p * W2:(p + 1) * W2],
                                            ALU.add)

                    # ---- gated conv2 + BN + PReLU + residual ----
                    g_ps = ps_pg.tile([C, W2], F32, tag="pg")
                    nc.tensor.matmul(g_ps[:], wg[:], out1[:],
                                     start=True, stop=True)
                    t2 = otp.tile([C, W2], BF, tag="t2")
                    nc.scalar.activation(t2[:], g_ps[:], AF.Prelu,
                                         bias=b3[:], scale=1.0, alpha=0.25)
                    of = otp.tile([C, W2], F32, tag="of")
                    nc.gpsimd.tensor_tensor(of[:], t2[:], out1[:], ALU.add)
                    nc.sync.dma_start(out_d[:, pc:pc + W2], of[:])

    nc.compile()
    return nc


def _fold_bn(w, g, b, m, v):
    """Fold inference BN into conv weight + bias. w: [out, in]."""
    s = g / np.sqrt(v + EPS)
    return w * s[:, None], b - m * s


def _prep_inputs(input, w_qkv, bn1_g, bn1_b, bn1_m, bn1_v, a1,
                 w_proj, bn2_g, bn2_b, bn2_m, bn2_v, a2,
                 w_g2, bn3_g, bn3_b, bn3_m, bn3_v, a3):
    bf16 = ml_dtypes.bfloat16
    w1, b1 = _fold_bn(np.asarray(w_qkv, np.float32), bn1_g, bn1_b, bn1_m, bn1_v)
    w2, b2 = _fold_bn(np.asarray(w_proj, np.float32), bn2_g, bn2_b, bn2_m, bn2_v)
    w3, b3 = _fold_bn(np.asarray(w_g2, np.float32), bn3_g, bn3_b, bn3_m, bn3_v)

    def pair_bias(b):  # [64] -> [128,1] tiled for the 2-row partition layout
        return np.tile(np.asarray(b, np.float32).reshape(-1, 1), (2, 1))

    consts = {
        "wq": np.ascontiguousarray(w1[0:CA].T.astype(bf16)),        # [128,64]
        "wk": np.ascontiguousarray(w1[CA:2 * CA].T.astype(bf16)),
        "wv": np.ascontiguousarray(w1[2 * CA:3 * CA].T.astype(bf16)),
        # [65,128]: row 0 zero (softmax-denominator passthrough row)
        "wp": np.ascontiguousarray(
            np.vstack([np.zeros((1, C), np.float32), w2.T]).astype(bf16)),
        "wg": np.ascontiguousarray(w3.T.astype(bf16)),              # [128,128]
        "bq": pair_bias(b1[0:CA]),
        "bk": pair_bias(b1[CA:2 * CA]),
        "bv": pair_bias(b1[2 * CA:3 * CA]),
        "b2": np.asarray(b2, np.float32).reshape(C, 1),
        "b3": np.asarray(b3, np.float32).reshape(C, 1),
        "ident": np.eye(C, dtype=np.float32).astype(bf16),
    }
    return consts


def run(inputs, n_pairs=H // 2, debug_dump=False, _raw=False):
    key = (n_pairs, debug_dump)
    if key not in _CACHE:
        _CACHE[key] = build(n_pairs, debug_dump)
    nc = _CACHE[key]
    consts = _prep_inputs(**inputs)
    x = np.asarray(inputs["input"], np.float32)
    rows = n_pairs * 2
    in_maps = []
    for b in range(N_CORES):
        m = dict(consts)
        m["x"] = np.ascontiguousarray(x[b, :, 0:rows, :].reshape(C, rows * W))
        in_maps.append(m)
    res = run_bass_kernel_spmd(nc, in_maps, list(range(N_CORES)))
    if _raw:
        return res
    out = np.stack([res.results[b]["out"].reshape(C, rows, W)
                    for b in range(N_CORES)])
    return out.astype(np.float32)


def kernel(**inputs) -> np.ndarray:
    return run(inputs, n_pairs=H // 2)


# revision 4
# speedup vs baseline: 3.6496x; 3.6496x over previous
"""Trainium2 Bass kernel for GCAFA block (conv1x1+BN+PReLU -> axial W attention
-> proj conv + residual -> gated conv + residual).

Sharding: batch B=8 across 8 NeuronCores (data parallel), params replicated.
All matmuls in bf16 with fp32 PSUM accumulation; output fp32.

v3: 4-row (2-pair) groups to batch ACT work; exp in one ACT per pair;
v-PReLU on DVE; cast + final residual add on GpSimd; persistent
ones-interleaved V^T buffers.
"""

import os
import sys

for _p in ("/opt/trn_rl_repo", "/root/.axon_site/_ro/trn_rl_repo"):
    if os.path.isdir(_p) and _p not in sys.path:
        sys.path.insert(0, _p)

import numpy as np
import ml_dtypes

import concourse.bacc as bacc
import concourse.tile as tile
from concourse import mybir
from concourse.bass_utils import run_bass_kernel_spmd

B, C, H, W = 8, 128, 224, 224
CA = C // 2  # 64
EPS = 1e-5
N_CORES = 8
PIX = H * W

F32 = mybir.dt.float32
BF = mybir.dt.bfloat16
AF = mybir.ActivationFunctionType
ALU = mybir.AluOpType

_CACHE = {}

N_VTS = 8  # persistent rotating V^T buffers (ones pre-interleaved)


def build(n_pairs=H // 2, debug_dump=False):
    """Build + compile the per-core Bass program processing 2*n_pairs rows."""
    assert n_pairs % 2 == 0
    n_groups = n_pairs // 2
    nc = bacc.Bacc("TRN2", target_bir_lowering=False, debug=False,
                   num_devices=N_CORES)
    npx = n_pairs * 2 * W  # pixels processed

    x_d = nc.dram_tensor("x", [C, npx], F32, kind="ExternalInput").ap()
    out_d = nc.dram_tensor("out", [C, npx], F32, kind="ExternalOutput").ap()
    wq_d = nc.dram_tensor("wq", [C, CA], BF, kind="ExternalInput").ap()
    wk_d = nc.dram_tensor("wk", [C, CA], BF, kind="ExternalInput").ap()
    wv_d = nc.dram_tensor("wv", [C, CA], BF, kind="ExternalInput").ap()
    wp_d = nc.dram_tensor("wp", [CA + 1, C], BF, kind="ExternalInput").ap()
    wg_d = nc.dram_tensor("wg", [C, C], BF, kind="ExternalInput").ap()
    bq_d = nc.dram_tensor("bq", [C, 1], F32, kind="ExternalInput").ap()
    bk_d = nc.dram_tensor("bk", [C, 1], F32, kind="ExternalInput").ap()
    bv_d = nc.dram_tensor("bv", [C, 1], F32, kind="ExternalInput").ap()
    b2_d = nc.dram_tensor("b2", [C, 1], F32, kind="ExternalInput").ap()
    b3_d = nc.dram_tensor("b3", [C, 1], F32, kind="ExternalInput").ap()
    id_d = nc.dram_tensor("ident", [C, C], BF, kind="ExternalInput").ap()

    W2 = 2 * W    # 448 = one pair of rows
    W4 = 4 * W    # 896 = one group (2 pairs)

    with tile.TileContext(nc) as tc:
        with (
            tc.tile_pool(name="consts", bufs=1) as cpool,
            tc.tile_pool(name="io", bufs=3) as iop,
            tc.tile_pool(name="acts", bufs=2) as ap_,
            tc.tile_pool(name="attn", bufs=3) as atp,
            tc.tile_pool(name="outs", bufs=3) as otp,
            tc.tile_pool(name="ps_qkv", bufs=1, space="PSUM") as ps_qkv,
            tc.tile_pool(name="ps_st", bufs=1, space="PSUM") as ps_st,
            tc.tile_pool(name="ps_vt", bufs=1, space="PSUM") as ps_vt,
            tc.tile_pool(name="ps_o", bufs=1, space="PSUM") as ps_o,
            tc.tile_pool(name="ps_pg", bufs=1, space="PSUM") as ps_pg,
        ):
            # ---- constants (loaded once) ----
            wq = cpool.tile([C, CA], BF, tag="wq")
            wk = cpool.tile([C, CA], BF, tag="wk")
            wv = cpool.tile([C, CA], BF, tag="wv")
            wp = cpool.tile([CA + 1, C], BF, tag="wp")
            wg = cpool.tile([C, C], BF, tag="wg")
            bq = cpool.tile([C, 1], F32, tag="bq")
            bk = cpool.tile([C, 1], F32, tag="bk")
            bv = cpool.tile([C, 1], F32, tag="bv")
            b2 = cpool.tile([C, 1], F32, tag="b2")
            b3 = cpool.tile([C, 1], F32, tag="b3")
            ident = cpool.tile([C, C], BF, tag="id")
            for t, d in ((wq, wq_d), (wk, wk_d), (wv, wv_d), (wp, wp_d),
                         (wg, wg_d), (bq, bq_d), (bk, bk_d), (bv, bv_d),
                         (b2, b2_d), (b3, b3_d), (ident, id_d)):
                nc.sync.dma_start(t[:], d[:])

            # persistent V^T buffers: per 66-col chunk [pad, ones, V^T(64)];
            # memset to 1.0 once, V-slots overwritten each use, ones survive.
            vts_bufs = []
            for i in range(N_VTS):
                vt_t = cpool.tile([112, 132], BF, tag=f"vts{i}")
                nc.gpsimd.memset(vt_t[:], 1.0)
                vts_bufs.append(vt_t)

            for g in range(n_groups):
                c0 = g * W4
                # ---- load + cast input group (4 rows) ----
                xf = iop.tile([C, W4], F32, tag="xf")
                nc.sync.dma_start(xf[:], x_d[:, c0:c0 + W4])
                xb = iop.tile([C, W4], BF, tag="xb")
                nc.vector.tensor_copy(xb[:], xf[:])

                # ---- qkv convs for the group ----
                # tiles [128, 448]: partitions = 2 rows x 64 chans
                # (row even -> parts 0:64, row odd -> 64:128),
                # cols = pair index * 224 + pixel
                q_ps = ps_qkv.tile([C, W2], F32, tag="q")
                k_ps = ps_qkv.tile([C, W2], F32, tag="k")
                v_ps = ps_qkv.tile([C, W2], F32, tag="v")
                for wmat, dst in ((wq, q_ps), (wk, k_ps), (wv, v_ps)):
                    for p in range(2):
                        for r in range(2):
                            rs = slice((2 * p + r) * W, (2 * p + r + 1) * W)
                            od = slice(r * CA, (r + 1) * CA)
                            cs = slice(p * W, (p + 1) * W)
                            nc.tensor.matmul(dst[od, cs], wmat[:], xb[:, rs],
                                             start=True, stop=True,
                                             tile_position=(0, r * CA))
                # q/k PReLU on ACT (one instr per tensor, whole group)
                qsb = ap_.tile([C, W2], BF, tag="q")
                ksb = ap_.tile([C, W2], BF, tag="k")
                nc.scalar.activation(qsb[:], q_ps[:], AF.Prelu,
                                     bias=bq[:], scale=1.0, alpha=0.25)
                nc.scalar.activation(ksb[:], k_ps[:], AF.Prelu,
                                     bias=bk[:], scale=1.0, alpha=0.25)
                # v PReLU on DVE: t = v + bv; A = relu(t); B = t - A = min(t,0)
                # vsb = 0.25*B + A
                va = ap_.tile([C, W2], BF, tag="va")
                vb = ap_.tile([C, W2], BF, tag="vb")
                vsb = ap_.tile([C, W2], BF, tag="v")
                nc.vector.tensor_scalar(va[:], v_ps[:], bv[:], 0.0,
                                        op0=ALU.add, op1=ALU.max)
                nc.vector.scalar_tensor_tensor(vb[:], v_ps[:], bv[:], va[:],
                                               op0=ALU.add, op1=ALU.subtract)
                nc.vector.scalar_tensor_tensor(vsb[:], vb[:], 0.25, va[:],
                                               op0=ALU.mult, op1=ALU.add)

                for p in range(2):
                    pr = 2 * g + p          # global pair index
                    po = p * W              # pair col offset in group tiles
                    pc = c0 + p * W2        # pair col offset in DRAM

                    # ---- scores: st [112, 1024]; row h at cols 0:448,
                    # row h+1 at cols 512:960 (bank-aligned chunks) ----
                    st = ps_st.tile([112, 1024], F32, tag="st")
                    for r in range(2):
                        part = slice(r * CA, (r + 1) * CA)
                        ro = r * 512
                        nc.tensor.matmul(
                            st[:, ro:ro + W], ksb[part, po:po + 112],
                            qsb[part, po:po + W],
                            start=True, stop=True, tile_position=(r * CA, 0))
                        nc.tensor.matmul(
                            st[:, ro + W:ro + W2], ksb[part, po + 112:po + W],
                            qsb[part, po:po + W],
                            start=True, stop=True, tile_position=(r * CA, 0))
                    # exp over the whole tile (junk cols harmless)
                    esb = atp.tile([112, 1024], BF, tag="e")
                    nc.scalar.activation(esb[:], st[:], AF.Exp,
                                         bias=0.0, scale=0.125)

                    # ---- V^T via PE transpose + interleave copy ----
                    vt_ps = ps_vt.tile([112, 2 * C], BF, tag="vt")
                    nc.tensor.transpose(vt_ps[:, 0:C],
                                        vsb[:, po:po + 112], ident[:])
                    nc.tensor.transpose(vt_ps[:, C:2 * C],
                                        vsb[:, po + 112:po + W], ident[:])
                    vtg = vt_ps[:].rearrange("p (c x) -> p c x", x=CA)
                    vts0 = vts_bufs[(2 * pr) % N_VTS]
                    vts1 = vts_bufs[(2 * pr + 1) % N_VTS]
                    for r, vts in ((0, vts0), (1, vts1)):
                        vtv = vts[:].rearrange("p (c x) -> p c x", x=CA + 2)
                        nc.vector.tensor_copy(vtv[:, :, 2:CA + 2],
                                              vtg[:, r::2, :])

                    # ---- PV: O' [65, 448]; row 0 = softmax denominator ----
                    o_ps = ps_o.tile([CA + 1, W2], F32, tag="o")
                    for r, vts in ((0, vts0), (1, vts1)):
                        ow = slice(r * W, (r + 1) * W)
                        eo = r * 512
                        nc.tensor.matmul(o_ps[:, ow], vts[:, 1:CA + 2],
                                         esb[:, eo:eo + W],
                                         start=True, stop=False)
                        nc.tensor.matmul(o_ps[:, ow], vts[:, CA + 3:2 * CA + 4],
                                         esb[:, eo + W:eo + W2],
                                         start=False, stop=True)

                    # ---- normalize: obar = O' * (1/denom) ----
                    rden = atp.tile([1, W2], F32, tag="rden")
                    nc.vector.reciprocal_approx_fast(rden[:], o_ps[0:1, :])
                    rbc = atp.tile([CA + 1, W2], F32, tag="rbc")
                    nc.gpsimd.partition_broadcast(rbc[:], rden[:])
                    obar = atp.tile([CA + 1, W2], BF, tag="obar")
                    nc.vector.tensor_tensor(obar[:], o_ps[:], rbc[:], ALU.mult)

                    # ---- proj conv + BN + PReLU + residual ----
                    pj_ps = ps_pg.tile([C, W2], F32, tag="pg")
                    nc.tensor.matmul(pj_ps[:], wp[:], obar[:],
                                     start=True, stop=True)
                    t1 = otp.tile([C, W2], BF, tag="t1")
                    nc.scalar.activation(t1[:], pj_ps[:], AF.Prelu,
                                         bias=b2[:], scale=1.0, alpha=0.25)
                    out1 = otp.tile([C, W2], BF, tag="out1")
                    nc.vector.tensor_tensor(out1[:], t1[:],
                                            xb[:, p * W2:(p + 1) * W2],
                                            ALU.add)

                    # ---- gated conv2 + BN + PReLU + residual ----
                    g_ps = ps_pg.tile([C, W2], F32, tag="pg")
                    nc.tensor.matmul(g_ps[:], wg[:], out1[:],
                                     start=True, stop=True)
                    t2 = otp.tile([C, W2], BF, tag="t2")
                    nc.scalar.activation(t2[:], g_ps[:], AF.Prelu,
                                         bias=b3[:], scale=1.0, alpha=0.25)
                    of = otp.tile([C, W2], F32, tag="of")
                    nc.vector.tensor_tensor(of[:], t2[:], out1[:], ALU.add)
                    nc.sync.dma_start(out_d[:, pc:pc + W2], of[:])

    nc.compile()
    return nc


def _fold_bn(w, g, b, m, v):
    """Fold inference BN into conv weight + bias. w: [out, in]."""
    s = g / np.sqrt(v + EPS)
    return w * s[:, None], b - m * s


def _prep_inputs(input, w_qkv, bn1_g, bn1_b, bn1_m, bn1_v, a1,
                 w_proj, bn2_g, bn2_b, bn2_m, bn2_v, a2,
                 w_g2, bn3_g, bn3_b, bn3_m, bn3_v, a3):
    bf16 = ml_dtypes.bfloat16
    w1, b1 = _fold_bn(np.asarray(w_qkv, np.float32), bn1_g, bn1_b, bn1_m, bn1_v)
    w2, b2 = _fold_bn(np.asarray(w_proj, np.float32), bn2_g, bn2_b, bn2_m, bn2_v)
    w3, b3 = _fold_bn(np.asarray(w_g2, np.float32), bn3_g, bn3_b, bn3_m, bn3_v)

    def pair_bias(b):  # [64] -> [128,1] tiled for the 2-row partition layout
        return np.tile(np.asarray(b, np.float32).reshape(-1, 1), (2, 1))

    consts = {
        "wq": np.ascontiguousarray(w1[0:CA].T.astype(bf16)),        # [128,64]
        "wk": np.ascontiguousarray(w1[CA:2 * CA].T.astype(bf16)),
        "wv": np.ascontiguousarray(w1[2 * CA:3 * CA].T.astype(bf16)),
        # [65,128]: row 0 zero (softmax-denominator passthrough row)
        "wp": np.ascontiguousarray(
            np.vstack([np.zeros((1, C), np.float32), w2.T]).astype(bf16)),
        "wg": np.ascontiguousarray(w3.T.astype(bf16)),              # [128,128]
        "bq": pair_bias(b1[0:CA]),
        "bk": pair_bias(b1[CA:2 * CA]),
        "bv": pair_bias(b1[2 * CA:3 * CA]),
        "b2": np.asarray(b2, np.float32).reshape(C, 1),
        "b3": np.asarray(b3, np.float32).reshape(C, 1),
        "ident": np.eye(C, dtype=np.float32).astype(bf16),
    }
    return consts


def run(inputs, n_pairs=H // 2, debug_dump=False, _raw=False):
    key = (n_pairs, debug_dump)
    if key not in _CACHE:
        _CACHE[key] = build(n_pairs, debug_dump)
    nc = _CACHE[key]
    consts = _prep_inputs(**inputs)
    x = np.asarray(inputs["input"], np.float32)
    rows = n_pairs * 2
    in_maps = []
    for b in range(N_CORES):
        m = dict(consts)
        m["x"] = np.ascontiguousarray(x[b, :, 0:rows, :].reshape(C, rows * W))
        in_maps.append(m)
    res = run_bass_kernel_spmd(nc, in_maps, list(range(N_CORES)))
    if _raw:
        return res
    out = np.stack([res.results[b]["out"].reshape(C, rows, W)
                    for b in range(N_CORES)])
    return out.astype(np.float32)


def kernel(**inputs) -> np.ndarray:
    return run(inputs, n_pairs=H // 2)
